# revision 6
# baseline (speedup 1.0000x reference)
"""NeuralODE Euler-integration kernel for 8 TRN2 NeuronCores.

Problem: y' = MLP(y) with MLP = Linear(64,256)+ReLU, Linear(256,256)+ReLU,
Linear(256,64); Euler steps y_{t+1} = y_t + dt*MLP(y_t), T=200 steps total
(199 integration steps), B=4096, output [B, 200, 64] with slot 0 = y0.

Sharding: pure data-parallel. Each of the 8 cores owns 512 batch rows.
Weights are replicated. No inter-core communication.

On-device design (per core):
  - State kept transposed: [64 features (partitions), 512 batch (free)],
    in TWO copies: fp32 master y32 (exact Euler accumulation) and an fp32r
    shadow yr that feeds the next step's first matmul. Both are produced by
    parallel VectorE `scalar_tensor_tensor` ops reading the same PSUM bank:
        yr/y32 = (ps3 + dt*b3) + y32_prev
    (W3, b3 pre-scaled by dt on host, so ps3 = dt*W3^T h2.)
  - Weights pre-arranged on host so every matmul uses them as lhsT directly.
  - Matmuls run as float32r (full PE rate at free-dim>=256, ~TF32 precision,
    fp32 accumulate).
  - relu(x+b): chunk0 on ScalarE (activation bias), chunk1 on VectorE
    (tensor_scalar add+max) so both hidden chunks materialize in parallel.
  - Batch processed as `nsub` sub-batches (default 2x256) pipelined so PE
    keeps streaming while ACT/DVE work on the other sub-batch.
  - Per step, y32 DMA'd to HBM as out[t-1] ([199, 64, 512], t-major,
    feature-major, contiguous 2KB per partition). Host reassembles to
    [B, 200, 64].
"""
import numpy as np

import concourse.bass as bass
import concourse.tile as tile
from concourse import bacc, mybir
from concourse.bass_utils import run_bass_kernel_spmd

F32 = mybir.dt.float32
F32R = mybir.dt.float32r
RELU = mybir.ActivationFunctionType.Relu

B, D, H, T = 4096, 64, 256, 200
NCORES = 8
BL = B // NCORES          # 512 batch rows per core

_cache = {}


def build(nsteps: int, nsub: int = 2, precise: bool = True):
    """Build the per-core Bass program (same NEFF for all 8 cores)."""
    add = mybir.AluOpType.add
    mx = mybir.AluOpType.max
    FD = BL // nsub

    nc = bacc.Bacc("TRN2", target_bir_lowering=False, debug=False)
    y0r_d = nc.dram_tensor("y0Tr", [D, BL], F32R, kind="ExternalInput")
    y032_d = nc.dram_tensor("y0T32", [D, BL], F32, kind="ExternalInput")
    w1_d = nc.dram_tensor("w1", [D, 2, 128], F32R, kind="ExternalInput")
    w2_d = nc.dram_tensor("w2", [128, 2, 2, 128], F32R, kind="ExternalInput")
    w3_d = nc.dram_tensor("w3", [128, 2, D], F32R, kind="ExternalInput")
    b1_d = nc.dram_tensor("b1r", [128, 2], F32, kind="ExternalInput")
    b2_d = nc.dram_tensor("b2r", [128, 2], F32, kind="ExternalInput")
    b3_d = nc.dram_tensor("b3r", [D, 1], F32, kind="ExternalInput")
    out_d = nc.dram_tensor("out", [nsteps, D, BL], F32, kind="ExternalOutput")

    with tile.TileContext(nc) as tc:
        with tc.tile_pool(name="wpool", bufs=1) as wp, \
             tc.tile_pool(name="state", bufs=1) as sp, \
             tc.tile_pool(name="hpool", bufs=6) as hp, \
             tc.tile_pool(name="ps", bufs=8, space="PSUM") as pp:

            w1 = wp.tile([D, 2, 128], F32R)
            w2 = wp.tile([128, 2, 2, 128], F32R)
            w3 = wp.tile([128, 2, D], F32R)
            b1 = wp.tile([128, 2], F32)
            b2 = wp.tile([128, 2], F32)
            b3 = wp.tile([D, 1], F32)
            nc.sync.dma_start(w1[:], w1_d.ap())
            nc.sync.dma_start(w2[:], w2_d.ap())
            nc.sync.dma_start(w3[:], w3_d.ap())
            nc.sync.dma_start(b1[:], b1_d.ap())
            nc.sync.dma_start(b2[:], b2_d.ap())
            nc.sync.dma_start(b3[:], b3_d.ap())

            # rotating state buffers (WAR slack vs the out-DMA / next steps)
            NR = 3
            N32 = 3
            yrs = [sp.tile([D, BL], F32R, tag=f"yr{i}", name=f"yr{i}")
                   for i in range(NR)]
            y32s = [sp.tile([D, BL], F32, tag=f"y32{i}", name=f"y32{i}")
                    for i in range(N32)]
            nc.sync.dma_start(yrs[0][:], y0r_d.ap())
            if precise:
                nc.sync.dma_start(y32s[0][:], y032_d.ap())

            out_ap = out_d.ap()

            for t in range(1, nsteps + 1):
                srcr = yrs[(t - 1) % NR]
                dstr = yrs[t % NR]
                src32 = y32s[(t - 1) % N32]
                dst32 = y32s[t % N32]
                for s in range(nsub):
                    cs = bass.ts(s, FD)
                    # ---- layer 1: ps1 = W1^T y ----
                    ps1 = [pp.tile([128, FD], F32, tag="ps", name="ps1")
                           for _ in range(2)]
                    for mc in range(2):
                        nc.tensor.matmul(ps1[mc][:], w1[:, mc, :], srcr[:, cs],
                                         start=True, stop=True)
                    h1 = hp.tile([128, 2, FD], F32R, tag="h", name="h1")
                    nc.scalar.activation(h1[:, 0, :], ps1[0][:], RELU,
                                         bias=b1[:, 0:1], scale=1.0)
                    nc.vector.tensor_scalar(h1[:, 1, :], ps1[1][:],
                                            b1[:, 1:2], 0.0, op0=add, op1=mx)
                    # ---- layer 2: ps2 = W2^T h1 ----
                    ps2 = [pp.tile([128, FD], F32, tag="ps", name="ps2")
                           for _ in range(2)]
                    for mc in range(2):
                        for kc in range(2):
                            nc.tensor.matmul(ps2[mc][:], w2[:, kc, mc, :],
                                             h1[:, kc, :],
                                             start=(kc == 0), stop=(kc == 1))
                    h2 = hp.tile([128, 2, FD], F32R, tag="h", name="h2")
                    nc.scalar.activation(h2[:, 0, :], ps2[0][:], RELU,
                                         bias=b2[:, 0:1], scale=1.0)
                    nc.scalar.activation(h2[:, 1, :], ps2[1][:], RELU,
                                         bias=b2[:, 1:2], scale=1.0)
                    # ---- layer 3 + Euler update ----
                    ps3 = pp.tile([D, FD], F32, tag="ps", name="ps3")
                    for kc in range(2):
                        nc.tensor.matmul(ps3[:], w3[:, kc, :], h2[:, kc, :],
                                         start=(kc == 0), stop=(kc == 1))
                    if precise:
                        # critical: fp32r shadow feeds next step's matmul
                        nc.vector.scalar_tensor_tensor(dstr[:, cs], ps3[:],
                                                       b3[:, 0:1], src32[:, cs],
                                                       op0=add, op1=add)
                        # fp32 master keeps exact Euler accumulation
                        nc.vector.scalar_tensor_tensor(dst32[:, cs], ps3[:],
                                                       b3[:, 0:1], src32[:, cs],
                                                       op0=add, op1=add)
                    else:
                        nc.vector.scalar_tensor_tensor(dstr[:, cs], ps3[:],
                                                       b3[:, 0:1], srcr[:, cs],
                                                       op0=add, op1=add)
                if precise:
                    nc.sync.dma_start(out_ap[t - 1], dst32[:])
                else:
                    nc.sync.dma_start(out_ap[t - 1], dstr[:].bitcast(F32))
    nc.compile()
    return nc


def _prep_inputs(y0, t, W1, b1, W2, b2, W3, b3):
    dt = float(t[1] - t[0])
    w1r = np.ascontiguousarray(W1.reshape(D, 2, 128))
    w2r = np.ascontiguousarray(W2.reshape(2, 128, 2, 128).transpose(1, 0, 2, 3))
    w3r = np.ascontiguousarray((dt * W3).reshape(2, 128, D).transpose(1, 0, 2))
    b1r = np.ascontiguousarray(b1.reshape(2, 128).T)
    b2r = np.ascontiguousarray(b2.reshape(2, 128).T)
    b3r = np.ascontiguousarray((dt * b3).reshape(D, 1))
    in_maps = []
    for c in range(NCORES):
        y0T = np.ascontiguousarray(y0[c * BL:(c + 1) * BL].T)
        in_maps.append({"y0Tr": y0T, "y0T32": y0T, "w1": w1r, "w2": w2r,
                        "w3": w3r, "b1r": b1r, "b2r": b2r, "b3r": b3r})
    return in_maps


def kernel(y0, t, W1, b1, W2, b2, W3, b3, nsub: int = 2, precise: bool = True,
           **run_kwargs):
    nsteps = int(t.shape[0]) - 1
    key = (nsteps, nsub, precise)
    if key not in _cache:
        _cache[key] = build(nsteps, nsub, precise)
    nc = _cache[key]
    in_maps = _prep_inputs(y0, t, W1, b1, W2, b2, W3, b3)
    res = run_bass_kernel_spmd(nc, in_maps, core_ids=list(range(NCORES)),
                               **run_kwargs)
    # assemble [B, T, D]
    parts = []
    for c in range(NCORES):
        oc = res.results[c]["out"]            # [nsteps, D, BL]
        parts.append(np.ascontiguousarray(oc.transpose(2, 0, 1)))  # [BL, ns, D]
    full = np.concatenate(parts, axis=0)      # [B, nsteps, D]
    out = np.concatenate([y0[:, None, :].astype(np.float32), full], axis=1)
    return out


# revision 7
# speedup vs baseline: 1.1918x; 1.1918x over previous
"""NeuralODE Euler-integration kernel for 8 TRN2 NeuronCores.

Problem: y' = MLP(y) with MLP = Linear(64,256)+ReLU, Linear(256,256)+ReLU,
Linear(256,64); Euler steps y_{t+1} = y_t + dt*MLP(y_t), T=200 steps total
(199 integration steps), B=4096, output [B, 200, 64] with slot 0 = y0.

Sharding: pure data-parallel. Each of the 8 cores owns 512 batch rows.
Weights are replicated. No inter-core communication.

On-device design (per core):
  - State kept transposed: [64 features (partitions), 512 batch (free)],
    in TWO copies: fp32 master y32 (exact Euler accumulation) and an fp32r
    shadow yr that feeds the next step's first matmul. Both are produced by
    parallel VectorE `scalar_tensor_tensor` ops reading the same PSUM bank:
        yr/y32 = (ps3 + dt*b3) + y32_prev
    (W3, b3 pre-scaled by dt on host, so ps3 = dt*W3^T h2.)
  - Weights pre-arranged on host so every matmul uses them as lhsT directly.
  - Matmuls run as float32r (full PE rate at free-dim>=256, ~TF32 precision,
    fp32 accumulate).
  - relu(x+b): chunk0 on ScalarE (activation bias), chunk1 on VectorE
    (tensor_scalar add+max) so both hidden chunks materialize in parallel.
  - Batch processed as `nsub` sub-batches (default 2x256) pipelined so PE
    keeps streaming while ACT/DVE work on the other sub-batch.
  - Per step, y32 DMA'd to HBM as out[t-1] ([199, 64, 512], t-major,
    feature-major, contiguous 2KB per partition). Host reassembles to
    [B, 200, 64].
"""
import numpy as np

import concourse.bass as bass
import concourse.tile as tile
from concourse import bacc, mybir
from concourse.bass_utils import run_bass_kernel_spmd

F32 = mybir.dt.float32
F32R = mybir.dt.float32r
BF16 = mybir.dt.bfloat16
RELU = mybir.ActivationFunctionType.Relu

B, D, H, T = 4096, 64, 256, 200
NCORES = 8
BL = B // NCORES          # 512 batch rows per core

_cache = {}


def build(nsteps: int, nsub: int = 2, precise: bool = True,
          mmdt=F32R):
    """Build the per-core Bass program (same NEFF for all 8 cores)."""
    add = mybir.AluOpType.add
    mx = mybir.AluOpType.max
    FD = BL // nsub

    nc = bacc.Bacc("TRN2", target_bir_lowering=False, debug=False)
    y0r_d = nc.dram_tensor("y0Tr", [D, BL], mmdt, kind="ExternalInput")
    y032_d = nc.dram_tensor("y0T32", [D, BL], F32, kind="ExternalInput")
    w1_d = nc.dram_tensor("w1", [D, 2, 128], mmdt, kind="ExternalInput")
    w2_d = nc.dram_tensor("w2", [128, 2, 2, 128], mmdt, kind="ExternalInput")
    w3_d = nc.dram_tensor("w3", [128, 2, D], mmdt, kind="ExternalInput")
    b1_d = nc.dram_tensor("b1r", [128, 2], F32, kind="ExternalInput")
    b2_d = nc.dram_tensor("b2r", [128, 2], F32, kind="ExternalInput")
    b3_d = nc.dram_tensor("b3r", [D, 1], F32, kind="ExternalInput")
    out_d = nc.dram_tensor("out", [nsteps, D, BL], F32, kind="ExternalOutput")

    with tile.TileContext(nc) as tc:
        with tc.tile_pool(name="wpool", bufs=1) as wp, \
             tc.tile_pool(name="state", bufs=1) as sp, \
             tc.tile_pool(name="hpool", bufs=6) as hp, \
             tc.tile_pool(name="ps", bufs=8, space="PSUM") as pp:

            w1 = wp.tile([D, 2, 128], mmdt)
            w2 = wp.tile([128, 2, 2, 128], mmdt)
            w3 = wp.tile([128, 2, D], mmdt)
            b1 = wp.tile([128, 2], F32)
            b2 = wp.tile([128, 2], F32)
            b3 = wp.tile([D, 1], F32)
            nc.sync.dma_start(w1[:], w1_d.ap())
            nc.sync.dma_start(w2[:], w2_d.ap())
            nc.sync.dma_start(w3[:], w3_d.ap())
            nc.sync.dma_start(b1[:], b1_d.ap())
            nc.sync.dma_start(b2[:], b2_d.ap())
            nc.sync.dma_start(b3[:], b3_d.ap())

            # rotating state buffers (WAR slack vs the out-DMA / next steps)
            NR = 3
            N32 = 3
            yrs = [sp.tile([D, BL], mmdt, tag=f"yr{i}", name=f"yr{i}")
                   for i in range(NR)]
            y32s = [sp.tile([D, BL], F32, tag=f"y32{i}", name=f"y32{i}")
                    for i in range(N32)]
            nc.sync.dma_start(yrs[0][:], y0r_d.ap())
            if precise:
                nc.sync.dma_start(y32s[0][:], y032_d.ap())

            out_ap = out_d.ap()

            for t in range(1, nsteps + 1):
                srcr = yrs[(t - 1) % NR]
                dstr = yrs[t % NR]
                src32 = y32s[(t - 1) % N32]
                dst32 = y32s[t % N32]
                for s in range(nsub):
                    cs = bass.ts(s, FD)
                    # ---- layer 1: ps1 = W1^T y ----
                    ps1 = [pp.tile([128, FD], F32, tag="ps", name="ps1")
                           for _ in range(2)]
                    for mc in range(2):
                        nc.tensor.matmul(ps1[mc][:], w1[:, mc, :], srcr[:, cs],
                                         start=True, stop=True)
                    h1 = hp.tile([128, 2, FD], mmdt, tag="h", name="h1")
                    nc.scalar.activation(h1[:, 0, :], ps1[0][:], RELU,
                                         bias=b1[:, 0:1], scale=1.0)
                    nc.vector.tensor_scalar(h1[:, 1, :], ps1[1][:],
                                            b1[:, 1:2], 0.0, op0=add, op1=mx)
                    # ---- layer 2: ps2 = W2^T h1 ----
                    ps2 = [pp.tile([128, FD], F32, tag="ps", name="ps2")
                           for _ in range(2)]
                    for mc in range(2):
                        for kc in range(2):
                            nc.tensor.matmul(ps2[mc][:], w2[:, kc, mc, :],
                                             h1[:, kc, :],
                                             start=(kc == 0), stop=(kc == 1))
                    h2 = hp.tile([128, 2, FD], mmdt, tag="h", name="h2")
                    nc.scalar.activation(h2[:, 0, :], ps2[0][:], RELU,
                                         bias=b2[:, 0:1], scale=1.0)
                    nc.scalar.activation(h2[:, 1, :], ps2[1][:], RELU,
                                         bias=b2[:, 1:2], scale=1.0)
                    # ---- layer 3 + Euler update ----
                    ps3 = pp.tile([D, FD], F32, tag="ps", name="ps3")
                    for kc in range(2):
                        nc.tensor.matmul(ps3[:], w3[:, kc, :], h2[:, kc, :],
                                         start=(kc == 0), stop=(kc == 1))
                    if precise:
                        # critical: fp32r shadow feeds next step's matmul
                        nc.vector.scalar_tensor_tensor(dstr[:, cs], ps3[:],
                                                       b3[:, 0:1], src32[:, cs],
                                                       op0=add, op1=add)
                        # fp32 master keeps exact Euler accumulation
                        nc.vector.scalar_tensor_tensor(dst32[:, cs], ps3[:],
                                                       b3[:, 0:1], src32[:, cs],
                                                       op0=add, op1=add)
                    else:
                        nc.vector.scalar_tensor_tensor(dstr[:, cs], ps3[:],
                                                       b3[:, 0:1], srcr[:, cs],
                                                       op0=add, op1=add)
                if precise:
                    nc.sync.dma_start(out_ap[t - 1], dst32[:])
                else:
                    nc.sync.dma_start(out_ap[t - 1], dstr[:].bitcast(F32))
    nc.compile()
    return nc


def _prep_inputs(y0, t, W1, b1, W2, b2, W3, b3, npdt=np.float32):
    dt = float(t[1] - t[0])
    w1r = np.ascontiguousarray(W1.reshape(D, 2, 128))
    w2r = np.ascontiguousarray(W2.reshape(2, 128, 2, 128).transpose(1, 0, 2, 3))
    w3r = np.ascontiguousarray((dt * W3).reshape(2, 128, D).transpose(1, 0, 2))
    b1r = np.ascontiguousarray(b1.reshape(2, 128).T)
    b2r = np.ascontiguousarray(b2.reshape(2, 128).T)
    b3r = np.ascontiguousarray((dt * b3).reshape(D, 1))
    in_maps = []
    for c in range(NCORES):
        y0T = np.ascontiguousarray(y0[c * BL:(c + 1) * BL].T)
        in_maps.append({"y0Tr": y0T.astype(npdt), "y0T32": y0T,
                        "w1": w1r.astype(npdt), "w2": w2r.astype(npdt),
                        "w3": w3r.astype(npdt), "b1r": b1r, "b2r": b2r,
                        "b3r": b3r})
    return in_maps


def kernel(y0, t, W1, b1, W2, b2, W3, b3, nsub: int = 2, precise: bool = True,
           use_bf16: bool = True, **run_kwargs):
    nsteps = int(t.shape[0]) - 1
    key = (nsteps, nsub, precise, use_bf16)
    if key not in _cache:
        _cache[key] = build(nsteps, nsub, precise,
                            mmdt=BF16 if use_bf16 else F32R)
    nc = _cache[key]
    import ml_dtypes
    in_maps = _prep_inputs(y0, t, W1, b1, W2, b2, W3, b3,
                           npdt=ml_dtypes.bfloat16 if use_bf16 else np.float32)
    res = run_bass_kernel_spmd(nc, in_maps, core_ids=list(range(NCORES)),
                               **run_kwargs)
    # assemble [B, T, D]
    parts = []
    for c in range(NCORES):
        oc = res.results[c]["out"]            # [nsteps, D, BL]
        parts.append(np.ascontiguousarray(oc.transpose(2, 0, 1)))  # [BL, ns, D]
    full = np.concatenate(parts, axis=0)      # [B, nsteps, D]
    out = np.concatenate([y0[:, None, :].astype(np.float32), full], axis=1)
    return out


# revision 8
# speedup vs baseline: 1.2358x; 1.0370x over previous
"""NeuralODE Euler-integration kernel for 8 TRN2 NeuronCores.

Problem: y' = MLP(y) with MLP = Linear(64,256)+ReLU, Linear(256,256)+ReLU,
Linear(256,64); Euler steps y_{t+1} = y_t + dt*MLP(y_t), T=200 steps total
(199 integration steps), B=4096, output [B, 200, 64] with slot 0 = y0.

Sharding: pure data-parallel. Each of the 8 cores owns 512 batch rows.
Weights are replicated. No inter-core communication.

On-device design (per core):
  - State kept transposed: [64 features (partitions), 512 batch (free)],
    in TWO copies: fp32 master y32 (exact Euler accumulation) and an fp32r
    shadow yr that feeds the next step's first matmul. Both are produced by
    parallel VectorE `scalar_tensor_tensor` ops reading the same PSUM bank:
        yr/y32 = (ps3 + dt*b3) + y32_prev
    (W3, b3 pre-scaled by dt on host, so ps3 = dt*W3^T h2.)
  - Weights pre-arranged on host so every matmul uses them as lhsT directly.
  - Matmuls run as float32r (full PE rate at free-dim>=256, ~TF32 precision,
    fp32 accumulate).
  - relu(x+b): chunk0 on ScalarE (activation bias), chunk1 on VectorE
    (tensor_scalar add+max) so both hidden chunks materialize in parallel.
  - Batch processed as `nsub` sub-batches (default 2x256) pipelined so PE
    keeps streaming while ACT/DVE work on the other sub-batch.
  - Per step, y32 DMA'd to HBM as out[t-1] ([199, 64, 512], t-major,
    feature-major, contiguous 2KB per partition). Host reassembles to
    [B, 200, 64].
"""
import numpy as np

import concourse.bass as bass
import concourse.tile as tile
from concourse import bacc, mybir
from concourse.bass_utils import run_bass_kernel_spmd

F32 = mybir.dt.float32
F32R = mybir.dt.float32r
BF16 = mybir.dt.bfloat16
RELU = mybir.ActivationFunctionType.Relu

B, D, H, T = 4096, 64, 256, 200
NCORES = 8
BL = B // NCORES          # 512 batch rows per core

_cache = {}


def build(nsteps: int, nsub: int = 2, precise: bool = True,
          mmdt=F32R, nwarm: int = 0):
    """Build the per-core Bass program (same NEFF for all 8 cores)."""
    add = mybir.AluOpType.add
    mx = mybir.AluOpType.max
    FD = BL // nsub

    nc = bacc.Bacc("TRN2", target_bir_lowering=False, debug=False)
    y0r_d = nc.dram_tensor("y0Tr", [D, BL], mmdt, kind="ExternalInput")
    y032_d = nc.dram_tensor("y0T32", [D, BL], F32, kind="ExternalInput")
    w1_d = nc.dram_tensor("w1", [D, 2, 128], mmdt, kind="ExternalInput")
    w2_d = nc.dram_tensor("w2", [128, 2, 2, 128], mmdt, kind="ExternalInput")
    w3_d = nc.dram_tensor("w3", [128, 2, D], mmdt, kind="ExternalInput")
    b1_d = nc.dram_tensor("b1r", [128, 2], F32, kind="ExternalInput")
    b2_d = nc.dram_tensor("b2r", [128, 2], F32, kind="ExternalInput")
    b3_d = nc.dram_tensor("b3r", [D, 1], F32, kind="ExternalInput")
    out_d = nc.dram_tensor("out", [nsteps, D, BL], F32, kind="ExternalOutput")

    with tile.TileContext(nc) as tc:
        with tc.tile_pool(name="wpool", bufs=1) as wp, \
             tc.tile_pool(name="state", bufs=1) as sp, \
             tc.tile_pool(name="hpool", bufs=8) as hp, \
             tc.tile_pool(name="ps", bufs=7, space="PSUM") as pp, \
             tc.tile_pool(name="warm", bufs=1, space="PSUM") as wpp:

            w1 = wp.tile([D, 2, 128], mmdt)
            w2 = wp.tile([128, 2, 2, 128], mmdt)
            w3 = wp.tile([128, 2, D], mmdt)
            b1 = wp.tile([128, 2], F32)
            b2 = wp.tile([128, 2], F32)
            b3 = wp.tile([D, 1], F32)
            nc.sync.dma_start(w1[:], w1_d.ap())
            nc.sync.dma_start(w2[:], w2_d.ap())
            nc.sync.dma_start(w3[:], w3_d.ap())
            nc.sync.dma_start(b1[:], b1_d.ap())
            nc.sync.dma_start(b2[:], b2_d.ap())
            nc.sync.dma_start(b3[:], b3_d.ap())

            # rotating state buffers (WAR slack vs the out-DMA / next steps)
            NR = 4
            N32 = 4
            yrs = [sp.tile([D, BL], mmdt, tag=f"yr{i}", name=f"yr{i}")
                   for i in range(NR)]
            y32s = [sp.tile([D, BL], F32, tag=f"y32{i}", name=f"y32{i}")
                    for i in range(N32)]
            nc.sync.dma_start(yrs[0][:], y0r_d.ap())
            if precise:
                nc.sync.dma_start(y32s[0][:], y032_d.ap())

            out_ap = out_d.ap()

            if nwarm:
                wps = wpp.tile([128, 256], F32, name="warmps")
                wsrc = w2  # any resident bf16 SBUF data works as operands

            for t in range(1, nsteps + 1):
                srcr = yrs[(t - 1) % NR]
                dstr = yrs[t % NR]
                src32 = y32s[(t - 1) % N32]
                dst32 = y32s[t % N32]
                for w in range(nwarm):
                    nc.tensor.matmul(wps[:], wsrc[:, 0, 0, :],
                                     wsrc[:, 0, :, :].rearrange('p a b -> p (a b)'),
                                     start=True, stop=True, skip_group_check=True)
                for s in range(nsub):
                    cs = bass.ts(s, FD)
                    # ---- layer 1: ps1 = W1^T y ----
                    ps1 = [pp.tile([128, FD], F32, tag="ps", name="ps1")
                           for _ in range(2)]
                    for mc in range(2):
                        nc.tensor.matmul(ps1[mc][:], w1[:, mc, :], srcr[:, cs],
                                         start=True, stop=True)
                    h1 = hp.tile([128, 2, FD], mmdt, tag="h", name="h1")
                    nc.scalar.activation(h1[:, 0, :], ps1[0][:], RELU,
                                         bias=b1[:, 0:1], scale=1.0)
                    nc.vector.tensor_scalar(h1[:, 1, :], ps1[1][:],
                                            b1[:, 1:2], 0.0, op0=add, op1=mx)
                    # ---- layer 2: ps2 = W2^T h1 ----
                    ps2 = [pp.tile([128, FD], F32, tag="ps", name="ps2")
                           for _ in range(2)]
                    for mc in range(2):
                        for kc in range(2):
                            nc.tensor.matmul(ps2[mc][:], w2[:, kc, mc, :],
                                             h1[:, kc, :],
                                             start=(kc == 0), stop=(kc == 1))
                    h2 = hp.tile([128, 2, FD], mmdt, tag="h", name="h2")
                    nc.scalar.activation(h2[:, 0, :], ps2[0][:], RELU,
                                         bias=b2[:, 0:1], scale=1.0)
                    nc.scalar.activation(h2[:, 1, :], ps2[1][:], RELU,
                                         bias=b2[:, 1:2], scale=1.0)
                    # ---- layer 3 + Euler update ----
                    ps3 = pp.tile([D, FD], F32, tag="ps", name="ps3")
                    for kc in range(2):
                        nc.tensor.matmul(ps3[:], w3[:, kc, :], h2[:, kc, :],
                                         start=(kc == 0), stop=(kc == 1))
                    if precise:
                        # critical: fp32r shadow feeds next step's matmul
                        nc.vector.scalar_tensor_tensor(dstr[:, cs], ps3[:],
                                                       b3[:, 0:1], src32[:, cs],
                                                       op0=add, op1=add)
                        # fp32 master keeps exact Euler accumulation
                        nc.vector.scalar_tensor_tensor(dst32[:, cs], ps3[:],
                                                       b3[:, 0:1], src32[:, cs],
                                                       op0=add, op1=add)
                    else:
                        nc.vector.scalar_tensor_tensor(dstr[:, cs], ps3[:],
                                                       b3[:, 0:1], srcr[:, cs],
                                                       op0=add, op1=add)
                if precise:
                    nc.sync.dma_start(out_ap[t - 1], dst32[:])
                else:
                    nc.sync.dma_start(out_ap[t - 1], dstr[:].bitcast(F32))
    nc.compile()
    return nc


def _prep_inputs(y0, t, W1, b1, W2, b2, W3, b3, npdt=np.float32):
    dt = float(t[1] - t[0])
    w1r = np.ascontiguousarray(W1.reshape(D, 2, 128))
    w2r = np.ascontiguousarray(W2.reshape(2, 128, 2, 128).transpose(1, 0, 2, 3))
    w3r = np.ascontiguousarray((dt * W3).reshape(2, 128, D).transpose(1, 0, 2))
    b1r = np.ascontiguousarray(b1.reshape(2, 128).T)
    b2r = np.ascontiguousarray(b2.reshape(2, 128).T)
    b3r = np.ascontiguousarray((dt * b3).reshape(D, 1))
    in_maps = []
    for c in range(NCORES):
        y0T = np.ascontiguousarray(y0[c * BL:(c + 1) * BL].T)
        in_maps.append({"y0Tr": y0T.astype(npdt), "y0T32": y0T,
                        "w1": w1r.astype(npdt), "w2": w2r.astype(npdt),
                        "w3": w3r.astype(npdt), "b1r": b1r, "b2r": b2r,
                        "b3r": b3r})
    return in_maps


def kernel(y0, t, W1, b1, W2, b2, W3, b3, nsub: int = 2, precise: bool = True,
           use_bf16: bool = True, nwarm: int = 0, **run_kwargs):
    nsteps = int(t.shape[0]) - 1
    key = (nsteps, nsub, precise, use_bf16, nwarm)
    if key not in _cache:
        _cache[key] = build(nsteps, nsub, precise,
                            mmdt=BF16 if use_bf16 else F32R, nwarm=nwarm)
    nc = _cache[key]
    import ml_dtypes
    in_maps = _prep_inputs(y0, t, W1, b1, W2, b2, W3, b3,
                           npdt=ml_dtypes.bfloat16 if use_bf16 else np.float32)
    res = run_bass_kernel_spmd(nc, in_maps, core_ids=list(range(NCORES)),
                               **run_kwargs)
    # assemble [B, T, D]
    parts = []
    for c in range(NCORES):
        oc = res.results[c]["out"]            # [nsteps, D, BL]
        parts.append(np.ascontiguousarray(oc.transpose(2, 0, 1)))  # [BL, ns, D]
    full = np.concatenate(parts, axis=0)      # [B, nsteps, D]
    out = np.concatenate([y0[:, None, :].astype(np.float32), full], axis=1)
    return out
